# revision 10
# baseline (speedup 1.0000x reference)
"""Trainium2 Bass kernel for nn_Attention (B=4, S=2048, D=1024, H=16, hd=64, fp32).

Sharding (head-split tensor parallel + pairwise exchange): 8 cores; core c
handles batch b=c//2 and head-half hh=c%2 (8 heads). Each core computes
Q/K/V for its 8 heads over ALL 2048 rows of its batch (zero duplication of
the QKV projections), runs attention for its 8 heads over all queries, then
the two cores of a batch exchange attention outputs (bf16, 2MB, via two
pairwise AllGather collectives overlapped with attention) so each core can
run the full-rank output projection for its own 1024 query rows.

Per-core input x is permuted so the core's own query half comes first
(q_local 0:1023 = own rows on every core, keeping the program SPMD-
identical); all other per-core differences (weight slices, W_proj row
order, the AllGather-slot select mask) are carried in the input data.

Query slots of 512 are processed in order [2, 0, 3, 1] (peer-owned rows
first) so the exchanges fire early; slot-2/3 outputs are sent, slot-0/1
outputs are kept. The projection for own rows interleaves into the last
attention phases; only the final 4 row-chunks trail the last block.

Per-core pipeline (all matmuls bf16, accumulation fp32 in PSUM): identical
attention inner structure to the query-split kernel (2-heads-row-packed
scores, exp on ScalarE without max subtraction, ones-column in V giving the
softmax denominator, reciprocal off the critical path).
"""

import os
import sys

import numpy as np

B, S, D, H, HD = 4, 2048, 1024, 16, 64
JL = 4       # local head-pairs (8 heads per core)
OWN = 1024   # query rows owned per core
NC_ = 8

_cache = {}


def _build_nc():
    sys.path.insert(0, "/opt/trn_rl_repo")
    import concourse.bass as bass
    from concourse import bacc
    import concourse.mybir as mybir
    import concourse.tile as tile
    from concourse.masks import make_identity
    from contextlib import ExitStack

    F32 = mybir.dt.float32
    BF16 = mybir.dt.bfloat16
    MULT = mybir.AluOpType.mult
    ADD = mybir.AluOpType.add
    Exp = mybir.ActivationFunctionType.Exp

    SLOTS = (2, 0, 3, 1)  # query-slot processing order (peer rows first)

    nc = bacc.Bacc()
    x_d = nc.declare_dram_parameter("xb", [S, D], F32, isOutput=False)
    wq_d = nc.declare_dram_parameter("wq", [D, 512], F32, isOutput=False)
    wk_d = nc.declare_dram_parameter("wk", [D, 512], F32, isOutput=False)
    wv_d = nc.declare_dram_parameter("wv", [D, 512], F32, isOutput=False)
    wp_d = nc.declare_dram_parameter("wp", [D, D], F32, isOutput=False)
    bqp_d = nc.declare_dram_parameter("bqp", [128, JL], F32, isOutput=False)
    bkp_d = nc.declare_dram_parameter("bkp", [128, JL], F32, isOutput=False)
    bvr_d = nc.declare_dram_parameter("bvr", [1, 512], F32, isOutput=False)
    bpr_d = nc.declare_dram_parameter("bpr", [1, D], F32, isOutput=False)
    msel_d = nc.declare_dram_parameter("msel", [128, 2], F32, isOutput=False)
    out_d = nc.declare_dram_parameter("out", [OWN, D], F32, isOutput=True)

    with ExitStack() as ctx:
        tc = ctx.enter_context(tile.TileContext(nc))

        const = ctx.enter_context(tc.tile_pool(name="const", bufs=1))
        ident = const.tile([128, 128], BF16)
        make_identity(nc, ident[:, :])
        ones1 = const.tile([1, 128], BF16)
        nc.vector.memset(ones1[:, :], 1.0)
        bqp = const.tile([128, JL], F32)
        nc.sync.dma_start(out=bqp[:, :], in_=bqp_d[:, :])
        bkp = const.tile([128, JL], F32)
        nc.sync.dma_start(out=bkp[:, :], in_=bkp_d[:, :])
        bvr = const.tile([1, 512], BF16)
        nc.gpsimd.dma_start(out=bvr[:, :], in_=bvr_d[:, :])
        bpr = const.tile([1, D], BF16)
        nc.gpsimd.dma_start(out=bpr[:, :], in_=bpr_d[:, :])
        msel = const.tile([128, 2], F32)
        nc.sync.dma_start(out=msel[:, :], in_=msel_d[:, :])

        big = ctx.enter_context(tc.tile_pool(name="big", bufs=1))
        KT = big.tile([128, JL * S], BF16)      # [p(2 heads), (j, k)]
        QT = big.tile([128, JL * S], BF16)      # [p(2 heads), (j, q)]
        Vaug = big.tile([128, 16 * 8 * 65], BF16)  # [p(s%128), (st, h, 65)]
        outT = big.tile([128, JL * S], BF16)    # [p(2 heads d), (j, q)]
        oPeer = big.tile([128, JL * OWN], BF16)  # peer heads, own rows

        KTv = KT[:, :].rearrange("p (j k) -> p j k", j=JL)
        QTv = QT[:, :].rearrange("p (j q) -> p j q", j=JL)
        Vv = Vaug[:, :].rearrange("p (t h e) -> p t h e", t=16, h=8)
        oTv = outT[:, :].rearrange("p (j q) -> p j q", j=JL)
        oPv = oPeer[:, :].rearrange("p (j q) -> p j q", j=JL)

        nc.vector.memset(Vv[:, :, :, 64:65], 1.0)

        apool = ctx.enter_context(tc.tile_pool(name="att", bufs=4))
        npool = ctx.enter_context(tc.tile_pool(name="attn", bufs=1))
        xTp_cm = tc.tile_pool(name="xTp", bufs=1)
        xTp = xTp_cm.__enter__()
        xT = xTp.tile([128, 8 * S], BF16)      # [p, (dt, s)]
        xTv = xT[:, :].rearrange("p (d s) -> p d s", d=8)

        # DRAM bounce buffers for the pairwise output exchange
        dr = ctx.enter_context(tc.tile_pool(name="dr", bufs=1, space="DRAM"))
        ag_in = [dr.tile([JL, 128, 512], BF16, name=f"agin{i}")
                 for i in range(2)]
        ag_out = [dr.tile([2, JL, 128, 512], BF16, name=f"agout{i}")
                  for i in range(2)]

        psm = ctx.enter_context(tc.tile_pool(name="psm", bufs=2, space="PSUM"))
        pso = ctx.enter_context(tc.tile_pool(name="pso", bufs=2, space="PSUM"))

        # bias rows broadcast to all 128 partitions once
        bvb = const.tile([128, 512], BF16)
        bpb = const.tile([128, D], BF16)
        pbias = pso.tile([128, 512], F32, tag="pk", name="pbias")
        nc.tensor.matmul(pbias[:, :], ones1[:, :], bvr[:, :],
                         start=True, stop=True)
        nc.vector.tensor_copy(bvb[:, :], pbias[:, :])
        pbias2 = psm.tile([128, 1024], F32, tag="ps", name="pbias2")
        for nh in range(2):
            nc.tensor.matmul(pbias2[:, nh * 512:(nh + 1) * 512], ones1[:, :],
                             bpr[:, nh * 512:(nh + 1) * 512],
                             start=True, stop=True)
        nc.vector.tensor_copy(bpb[:, :], pbias2[:, :])

        wkq_cm = tc.tile_pool(name="wkq", bufs=1)
        wkq = wkq_cm.__enter__()

        def load_w_split(wd, pool, tag):
            tiles = [pool.tile([128, 512], BF16, tag=tag + "b" + str(dt_),
                               name=tag + str(dt_)) for dt_ in range(8)]
            for dt_ in range(8):  # j=0 columns first (unblocks K0/Q0)
                nc.gpsimd.dma_start(
                    out=tiles[dt_][:, 0:128],
                    in_=wd[dt_ * 128:(dt_ + 1) * 128, 0:128])
            return tiles

        def load_w_rest(wd, tiles):
            for dt_ in range(8):
                nc.gpsimd.dma_start(
                    out=tiles[dt_][:, 128:512],
                    in_=wd[dt_ * 128:(dt_ + 1) * 128, 128:512])

        wkb = None
        wqb = None

        def k_step(j, sc):
            pkc = pso.tile([128, 512], F32, tag="pk", name=f"pk{j}_{sc}")
            for dt_ in range(8):
                nc.tensor.matmul(
                    pkc[:, :],
                    wkb[dt_][:, j * 128:(j + 1) * 128],
                    xTv[:, dt_, sc * 512:(sc + 1) * 512],
                    start=(dt_ == 0), stop=(dt_ == 7),
                )
            nc.vector.tensor_scalar_add(
                KTv[:, j, sc * 512:(sc + 1) * 512], pkc[:, :],
                bkp[:, j:j + 1])

        def q_step(j, qs):
            pqc = pso.tile([128, 512], F32, tag="pk", name=f"pq{j}_{qs}")
            for dt_ in range(8):
                nc.tensor.matmul(
                    pqc[:, :],
                    wqb[dt_][:, j * 128:(j + 1) * 128],
                    xTv[:, dt_, qs * 512:(qs + 1) * 512],
                    start=(dt_ == 0), stop=(dt_ == 7),
                )
            nc.vector.tensor_scalar_add(
                QTv[:, j, qs * 512:(qs + 1) * 512], pqc[:, :],
                bqp[:, j:j + 1])

        def kq_chunks(j):
            # K (all 4 key chunks) + Q for the first-processed slot only;
            # Q chunks for later slots are deferred into later phases
            return [
                lambda j=j: k_step(j, 0),
                lambda j=j: q_step(j, SLOTS[0]),
                lambda j=j: k_step(j, 1),
                lambda j=j: k_step(j, 2),
                lambda j=j: k_step(j, 3),
            ]

        # ---------------- Phase A: xT via PE transposes ----------------
        # K0/Q0 chunk matmuls are emitted as soon as the x tiles they need
        # have been transposed, filling the PE while later x tiles stream in
        with tc.tile_pool(name="xstg", bufs=6) as xpool:
            for st in range(16):
                if st == 2:
                    wkb = load_w_split(wk_d, wkq, "wk")
                    wqb = load_w_split(wq_d, wkq, "wq")
                xb16 = xpool.tile([128, D], BF16, tag="xb16")
                nc.gpsimd.dma_start(out=xb16[:, :],
                                    in_=x_d[st * 128:(st + 1) * 128, :])
                pt = psm.tile([128, 1024], BF16, tag="ps", name=f"pt{st}")
                for dt_ in range(8):
                    nc.tensor.transpose(
                        pt[:, dt_ * 128:(dt_ + 1) * 128],
                        xb16[:, dt_ * 128:(dt_ + 1) * 128],
                        ident[:, :],
                    )
                dst = xTv[:, :, st * 128:(st + 1) * 128]
                src = pt[:, :].rearrange("p (d s) -> p d s", d=8)
                if st % 2 == 0:
                    nc.scalar.copy(dst, src)
                else:
                    nc.vector.tensor_copy(dst, src)
                if st % 4 == 3:
                    k_step(0, st // 4)          # needs x tiles 4sc..4sc+3
                    if st == 11:
                        q_step(0, SLOTS[0])     # slot-2 queries: tiles 8-11

        def load_w(wd, pool, tag, cols):
            tiles = []
            for dt_ in range(8):
                wb = pool.tile([128, cols], BF16, tag=tag + "b" + str(dt_))
                nc.gpsimd.dma_start(out=wb[:, :],
                                    in_=wd[dt_ * 128:(dt_ + 1) * 128, :])
                tiles.append(wb)
            return tiles

        rpbs = {}

        def attn_fast_evac(j, qs, poA, poB):
            qsl = slice(qs * 512, (qs + 1) * 512)
            lp = npool.tile([1, 1024], F32, tag="lp", name=f"lp{j}_{qs}")
            nc.vector.tensor_copy(lp[0:1, 0:512], poA[64:65, :])
            nc.vector.tensor_copy(lp[0:1, 512:1024], poB[64:65, :])
            nc.vector.tensor_copy(oTv[0:64, j, qsl], poA[0:64, :])
            nc.vector.tensor_copy(oTv[64:128, j, qsl], poB[0:64, :])
            rp = npool.tile([1, 1024], F32, tag="rp", name=f"rp{j}_{qs}")
            nc.vector.reciprocal_approx_fast(rp[:, :], lp[:, :])
            rpb = npool.tile([1, 1024], BF16, tag="rpb", name=f"rpb{j}_{qs}")
            nc.vector.tensor_copy(rpb[:, :], rp[:, :])
            rpbs[(j, qs)] = rpb

        def attn_norm_tail(j, qs):
            qsl = slice(qs * 512, (qs + 1) * 512)
            rpb = rpbs.pop((j, qs))
            pbc = pso.tile([128, 512], F32, tag="pk", name=f"pbc{j}_{qs}")
            nc.tensor.matmul(pbc[0:64, :], ones1[0:1, 0:64],
                             rpb[0:1, 0:512], start=True, stop=True)
            nc.tensor.matmul(pbc[64:128, :], ones1[0:1, 0:64],
                             rpb[0:1, 512:1024], start=True, stop=True,
                             tile_position=(0, 64))
            rbc = npool.tile([128, 512], F32, tag="rbc", name=f"rbc{j}_{qs}")
            nc.vector.tensor_copy(rbc[:, :], pbc[:, :])
            nc.vector.tensor_tensor(
                oTv[0:64, j, qsl], oTv[0:64, j, qsl], rbc[0:64, :], MULT)
            nc.vector.tensor_tensor(
                oTv[64:128, j, qsl], oTv[64:128, j, qsl], rbc[64:128, :],
                MULT)
            if qs == 2:
                nc.sync.dma_start(out=ag_in[0][j, :, :], in_=oTv[:, j, qsl])
            elif qs == 3:
                nc.sync.dma_start(out=ag_in[1][j, :, :], in_=oTv[:, j, qsl])

        pending_norm = []

        def flush_norm():
            while pending_norm:
                pending_norm.pop(0)()

        def attn_group(j, qs, ktg, poA, poB):
            qsl = slice(qs * 512, (qs + 1) * 512)
            kts = (2 * ktg, 2 * ktg + 1)
            pss = []
            for kt in kts:
                ps = psm.tile([128, 1024], F32, tag="ps",
                              name=f"ps{j}_{qs}_{kt}")
                pss.append(ps)
                nc.tensor.matmul(
                    ps[:, 0:512],
                    KTv[0:64, j, kt * 128:(kt + 1) * 128],
                    QTv[0:64, j, qsl],
                    start=True, stop=True, tile_position=(0, 0))
                nc.tensor.matmul(
                    ps[:, 512:1024],
                    KTv[64:128, j, kt * 128:(kt + 1) * 128],
                    QTv[64:128, j, qsl],
                    start=True, stop=True, tile_position=(64, 0))
            ePs = []
            for ps in pss:
                eP = apool.tile([128, 1024], BF16, tag="eP")
                nc.scalar.activation(eP[:, :], ps[:, :], Exp, scale=0.125)
                ePs.append(eP)
            for kt, eP in zip(kts, ePs):
                nc.tensor.matmul(
                    poA[:, :], Vv[:, kt, 2 * j, 0:65], eP[:, 0:512],
                    start=(kt == 0), stop=(kt == 15))
                nc.tensor.matmul(
                    poB[:, :], Vv[:, kt, 2 * j + 1, 0:65], eP[:, 512:1024],
                    start=(kt == 0), stop=(kt == 15))

        def attn_block(j, qs, interleave=None):
            poA = pso.tile([65, 512], F32, tag="po", name=f"poA{j}_{qs}")
            poB = pso.tile([65, 512], F32, tag="po", name=f"poB{j}_{qs}")
            nsteps = len(interleave) if interleave else 0
            si = 0
            for ktg in range(8):
                attn_group(j, qs, ktg, poA, poB)
                if interleave and si < nsteps and ktg < 6:
                    interleave[si]()
                    si += 1
                if ktg == 1:
                    flush_norm()
            while interleave and si < nsteps:
                interleave[si]()
                si += 1
            attn_fast_evac(j, qs, poA, poB)
            pending_norm.append(lambda j=j, qs=qs: attn_norm_tail(j, qs))

        # --- V proj pipelined with the first attention block (slot 2) ---
        with tc.tile_pool(name="wv", bufs=1) as wvp:
            wvb = load_w(wv_d, wvp, "wv", 512)
            load_w_rest(wk_d, wkb)
            load_w_rest(wq_d, wqb)

            def v_st(st):
                pv = psm.tile([128, 512], F32, tag="ps", name=f"pv{st}")
                for dt_ in range(8):
                    nc.tensor.matmul(
                        pv[:, :],
                        xTv[:, dt_, st * 128:(st + 1) * 128],
                        wvb[dt_][:, :],
                        start=(dt_ == 0), stop=(dt_ == 7),
                    )
                dst = Vv[:, st, :, 0:64]
                src_ = pv[:, :].rearrange("p (h d) -> p h d", h=8)
                bsr = bvb[:, :].rearrange("p (h d) -> p h d", h=8)
                nc.vector.tensor_tensor(dst, src_, bsr, ADD)

            poA0 = pso.tile([65, 512], F32, tag="po", name="poA0_s2")
            poB0 = pso.tile([65, 512], F32, tag="po", name="poB0_s2")
            for stg in range(8):
                v_st(2 * stg)
                v_st(2 * stg + 1)
                attn_group(0, 2, stg, poA0, poB0)
            attn_fast_evac(0, 2, poA0, poB0)
            pending_norm.append(lambda: attn_norm_tail(0, 2))

        # rest of slot-2 phase; block 3 hosts the deferred slot-0 Q chunks
        for step in kq_chunks(1):
            step()
        attn_block(1, 2, interleave=kq_chunks(2))
        attn_block(2, 2, interleave=kq_chunks(3))
        attn_block(3, 2,
                   interleave=[lambda j=j: q_step(j, SLOTS[1])
                               for j in range(JL)])
        flush_norm()

        # exchange #1: slot-2 outputs (this core's q_local 1024:1535)
        nc.gpsimd.collective_compute(
            "AllGather",
            mybir.AluOpType.bypass,
            replica_groups=[[0, 1], [2, 3], [4, 5], [6, 7]],
            ins=[ag_in[0][:, :, :]],
            outs=[ag_out[0][:, :, :, :]],
        )

        def consume_ag(i, qoff):
            stg = npool.tile([128, JL * 512], BF16, tag="agA",
                             name=f"agA{i}")
            stg2 = npool.tile([128, JL * 512], BF16, tag="agB",
                              name=f"agB{i}")
            sv = stg[:, :].rearrange("p (j q) -> p j q", j=JL)
            sv2 = stg2[:, :].rearrange("p (j q) -> p j q", j=JL)
            src0 = ag_out[i][0, :, :, :].rearrange("j p q -> p j q")
            src1 = ag_out[i][1, :, :, :].rearrange("j p q -> p j q")
            nc.sync.dma_start(out=sv, in_=src0)
            nc.sync.dma_start(out=sv2, in_=src1)
            dst = oPv[:, :, qoff:qoff + 512]
            # peer slot = slot1 on even cores (msel col0 = 1), slot0 on odd
            nc.vector.tensor_scalar_mul(dst, sv2, msel[:, 0:1])
            nc.vector.scalar_tensor_tensor(
                dst, sv, msel[:, 1:2], dst, op0=MULT, op1=ADD)

        consume_ag(0, 0)

        # ---------------- slot 0 (own rows, first half) ----------------
        # hosts the remaining deferred Q chunks (xT/wkq still alive here)
        for bj in range(JL):
            attn_block(bj, 0,
                       interleave=[lambda j=bj: q_step(j, SLOTS[2]),
                                   lambda j=bj: q_step(j, SLOTS[3])])
        flush_norm()
        wkq_cm.__exit__(None, None, None)
        xTp_cm.__exit__(None, None, None)

        # ---------------- slot 3 + proj(own first half) -----------------
        with tc.tile_pool(name="wp", bufs=1) as wpp, \
             tc.tile_pool(name="ystg", bufs=2) as ypool:
            wpb = load_w(wp_d, wpp, "wp", D)

            def proj_lhs(c, qt):
                qsl = slice(qt * 128, (qt + 1) * 128)
                if c < 4:
                    return oTv[:, c, qsl]
                return oPv[:, c - 4, qsl]

            def proj(qt):
                py = psm.tile([128, 1024], F32, tag="ps", name=f"py{qt}")
                for c in range(8):
                    for nh in range(2):
                        nc.tensor.matmul(
                            py[:, nh * 512:(nh + 1) * 512],
                            proj_lhs(c, qt),
                            wpb[c][:, nh * 512:(nh + 1) * 512],
                            start=(c == 0), stop=(c == 7),
                        )
                ys = ypool.tile([128, 1024], F32, tag="ys")
                nc.vector.tensor_tensor(ys[:, :], py[:, :], bpb[:, :], ADD)
                nc.sync.dma_start(
                    out=out_d[qt * 128:(qt + 1) * 128, :], in_=ys[:, :])

            def proj_nh(qt, nh):
                ph = pso.tile([128, 512], F32, tag="pk",
                              name=f"ph{qt}_{nh}")
                for c in range(8):
                    nc.tensor.matmul(
                        ph[:, :],
                        proj_lhs(c, qt),
                        wpb[c][:, nh * 512:(nh + 1) * 512],
                        start=(c == 0), stop=(c == 7),
                    )
                ys = ypool.tile([128, 512], F32, tag="ysh")
                nc.vector.tensor_tensor(
                    ys[:, :], ph[:, :], bpb[:, nh * 512:(nh + 1) * 512],
                    ADD)
                nc.sync.dma_start(
                    out=out_d[qt * 128:(qt + 1) * 128,
                              nh * 512:(nh + 1) * 512], in_=ys[:, :])

            # slot 3 blocks: host proj halves for qt 0-1 (deps: slot-0 attn
            # + exchange #1); qt 2-3 halves go into the slot-1 blocks
            halves = [(qt, nh) for qt in range(4) for nh in range(2)]
            for bj in range(JL):
                il = [lambda qt=qt, nh=nh: proj_nh(qt, nh)
                      for (qt, nh) in halves[bj:bj + 1]]
                attn_block(bj, 3, interleave=il)
            flush_norm()

            # exchange #2: slot-3 outputs
            nc.gpsimd.collective_compute(
                "AllGather",
                mybir.AluOpType.bypass,
                replica_groups=[[0, 1], [2, 3], [4, 5], [6, 7]],
                ins=[ag_in[1][:, :, :]],
                outs=[ag_out[1][:, :, :, :]],
            )
            consume_ag(1, 512)

            # ---------------- slot 1 (own rows, second half) -------------
            for bj in range(JL):
                il = [lambda qt=qt, nh=nh: proj_nh(qt, nh)
                      for (qt, nh) in halves[4 + bj:5 + bj]]
                attn_block(bj, 1, interleave=il)

            # tail: qt=4's j!=3 chunks overlap the last block's norm chain
            py4 = psm.tile([128, 1024], F32, tag="ps", name="py4")
            for c in (0, 1, 2, 4, 5, 6, 7):
                for nh in range(2):
                    nc.tensor.matmul(
                        py4[:, nh * 512:(nh + 1) * 512],
                        proj_lhs(c, 4),
                        wpb[c][:, nh * 512:(nh + 1) * 512],
                        start=(c == 0), stop=False,
                    )
            flush_norm()
            for nh in range(2):
                nc.tensor.matmul(
                    py4[:, nh * 512:(nh + 1) * 512],
                    proj_lhs(3, 4),
                    wpb[3][:, nh * 512:(nh + 1) * 512],
                    start=False, stop=True,
                )
            ys4 = ypool.tile([128, 1024], F32, tag="ys")
            nc.vector.tensor_tensor(ys4[:, :], py4[:, :], bpb[:, :], ADD)
            nc.sync.dma_start(out=out_d[4 * 128:5 * 128, :], in_=ys4[:, :])
            for qt in range(5, 8):
                proj(qt)

    nc.finalize()
    return nc


def _in_maps(x, W_qkv, b_qkv, W_proj, b_proj):
    x = np.asarray(x, np.float32)
    W_qkv = np.asarray(W_qkv, np.float32)
    b_qkv = np.asarray(b_qkv, np.float32)
    W_proj = np.ascontiguousarray(np.asarray(W_proj, np.float32))
    b_proj = np.asarray(b_proj, np.float32)
    Wq = W_qkv[:, 0:D]
    Wk = W_qkv[:, D:2 * D]
    Wv = W_qkv[:, 2 * D:3 * D]
    bq, bk = b_qkv[0:D], b_qkv[D:2 * D]
    bv = b_qkv[2 * D:3 * D]
    maps = []
    for c in range(NC_):
        b, hh = c // 2, c % 2
        hs = slice(hh * 512, (hh + 1) * 512)
        xb = np.concatenate(
            [x[b, hh * OWN:(hh + 1) * OWN],
             x[b, (1 - hh) * OWN:(2 - hh) * OWN]], axis=0)
        wp = np.concatenate(
            [W_proj[hh * 512:(hh + 1) * 512],
             W_proj[(1 - hh) * 512:(2 - hh) * 512]], axis=0)
        msel = np.zeros((128, 2), np.float32)
        msel[:, 0] = 1.0 if hh == 0 else 0.0
        msel[:, 1] = 1.0 - msel[:, 0]
        maps.append({
            "xb": np.ascontiguousarray(xb),
            "wq": np.ascontiguousarray(Wq[:, hs]),
            "wk": np.ascontiguousarray(Wk[:, hs]),
            "wv": np.ascontiguousarray(Wv[:, hs]),
            "wp": np.ascontiguousarray(wp),
            "bqp": np.ascontiguousarray(bq[hs].reshape(JL, 128).T),
            "bkp": np.ascontiguousarray(bk[hs].reshape(JL, 128).T),
            "bvr": np.ascontiguousarray(bv[hs].reshape(1, 512)),
            "bpr": np.ascontiguousarray(b_proj.reshape(1, D)),
            "msel": msel,
        })
    return maps


def run(x, W_qkv, b_qkv, W_proj, b_proj, trace=False, tmpdir=None):
    sys.path.insert(0, "/opt/trn_rl_repo")
    from concourse.bass_utils import run_bass_kernel_spmd

    if "nc" not in _cache:
        _cache["nc"] = _build_nc()
    nc = _cache["nc"]
    maps = _in_maps(x, W_qkv, b_qkv, W_proj, b_proj)
    res = run_bass_kernel_spmd(nc, maps, core_ids=list(range(NC_)),
                               trace=trace, tmpdir=tmpdir)
    y = np.empty((B, S, D), np.float32)
    for c in range(NC_):
        b, hh = c // 2, c % 2
        y[b, hh * OWN:(hh + 1) * OWN] = res.results[c]["out"]
    return y, res


def kernel(x, W_qkv, b_qkv, W_proj, b_proj):
    y, _ = run(x, W_qkv, b_qkv, W_proj, b_proj, trace=False)
    return y


# revision 12
# speedup vs baseline: 1.0282x; 1.0282x over previous
"""Trainium2 Bass kernel for nn_Attention (B=4, S=2048, D=1024, H=16, hd=64, fp32).

Sharding (head-split tensor parallel + pairwise exchange): 8 cores; core c
handles batch b=c//2 and head-half hh=c%2 (8 heads). Each core computes
Q/K/V for its 8 heads over ALL 2048 rows of its batch (zero duplication of
the QKV projections), runs attention for its 8 heads over all queries, then
the two cores of a batch exchange attention outputs (bf16, 2MB, via two
pairwise AllGather collectives overlapped with attention) so each core can
run the full-rank output projection for its own 1024 query rows.

Per-core input x is permuted so the core's own query half comes first
(q_local 0:1023 = own rows on every core, keeping the program SPMD-
identical); all other per-core differences (weight slices, W_proj row
order, the AllGather-slot select mask) are carried in the input data.

Query slots of 512 are processed in order [2, 0, 3, 1] (peer-owned rows
first) so the exchanges fire early; slot-2/3 outputs are sent, slot-0/1
outputs are kept. The projection for own rows interleaves into the last
attention phases; only the final 4 row-chunks trail the last block.

Per-core pipeline (all matmuls bf16, accumulation fp32 in PSUM): identical
attention inner structure to the query-split kernel (2-heads-row-packed
scores, exp on ScalarE without max subtraction, ones-column in V giving the
softmax denominator, reciprocal off the critical path).
"""

import os
import sys

import numpy as np

B, S, D, H, HD = 4, 2048, 1024, 16, 64
JL = 4       # local head-pairs (8 heads per core)
OWN = 1024   # query rows owned per core
NC_ = 8

_cache = {}


def _build_nc():
    sys.path.insert(0, "/opt/trn_rl_repo")
    import concourse.bass as bass
    from concourse import bacc
    import concourse.mybir as mybir
    import concourse.tile as tile
    from concourse.masks import make_identity
    from contextlib import ExitStack

    F32 = mybir.dt.float32
    BF16 = mybir.dt.bfloat16
    MULT = mybir.AluOpType.mult
    ADD = mybir.AluOpType.add
    Exp = mybir.ActivationFunctionType.Exp

    SLOTS = (2, 0, 3, 1)  # query-slot processing order (peer rows first)

    nc = bacc.Bacc()
    x_d = nc.declare_dram_parameter("xb", [S, D], F32, isOutput=False)
    wq_d = nc.declare_dram_parameter("wq", [D, 512], F32, isOutput=False)
    wk_d = nc.declare_dram_parameter("wk", [D, 512], F32, isOutput=False)
    wv_d = nc.declare_dram_parameter("wv", [D, 512], F32, isOutput=False)
    wp_d = nc.declare_dram_parameter("wp", [D, D], F32, isOutput=False)
    bqp_d = nc.declare_dram_parameter("bqp", [128, JL], F32, isOutput=False)
    bkp_d = nc.declare_dram_parameter("bkp", [128, JL], F32, isOutput=False)
    bvr_d = nc.declare_dram_parameter("bvr", [1, 512], F32, isOutput=False)
    bpr_d = nc.declare_dram_parameter("bpr", [1, D], F32, isOutput=False)
    msel_d = nc.declare_dram_parameter("msel", [128, 2], F32, isOutput=False)
    out_d = nc.declare_dram_parameter("out", [OWN, D], F32, isOutput=True)

    with ExitStack() as ctx:
        tc = ctx.enter_context(tile.TileContext(nc))

        const = ctx.enter_context(tc.tile_pool(name="const", bufs=1))
        ident = const.tile([128, 128], BF16)
        make_identity(nc, ident[:, :])
        ones1 = const.tile([1, 128], BF16)
        nc.vector.memset(ones1[:, :], 1.0)
        bqp = const.tile([128, JL], F32)
        nc.sync.dma_start(out=bqp[:, :], in_=bqp_d[:, :])
        bkp = const.tile([128, JL], F32)
        nc.sync.dma_start(out=bkp[:, :], in_=bkp_d[:, :])
        bvr = const.tile([1, 512], BF16)
        nc.gpsimd.dma_start(out=bvr[:, :], in_=bvr_d[:, :])
        bpr = const.tile([1, D], BF16)
        nc.gpsimd.dma_start(out=bpr[:, :], in_=bpr_d[:, :])
        msel = const.tile([128, 2], F32)
        nc.sync.dma_start(out=msel[:, :], in_=msel_d[:, :])

        big = ctx.enter_context(tc.tile_pool(name="big", bufs=1))
        KT = big.tile([128, JL * S], BF16)      # [p(2 heads), (j, k)]
        QT = big.tile([128, JL * S], BF16)      # [p(2 heads), (j, q)]
        Vaug = big.tile([128, 16 * 8 * 65], BF16)  # [p(s%128), (st, h, 65)]
        outT = big.tile([128, JL * S], BF16)    # [p(2 heads d), (j, q)]
        oPeer = big.tile([128, JL * OWN], BF16)  # peer heads, own rows

        KTv = KT[:, :].rearrange("p (j k) -> p j k", j=JL)
        QTv = QT[:, :].rearrange("p (j q) -> p j q", j=JL)
        Vv = Vaug[:, :].rearrange("p (t h e) -> p t h e", t=16, h=8)
        oTv = outT[:, :].rearrange("p (j q) -> p j q", j=JL)
        oPv = oPeer[:, :].rearrange("p (j q) -> p j q", j=JL)

        nc.vector.memset(Vv[:, :, :, 64:65], 1.0)

        apool = ctx.enter_context(tc.tile_pool(name="att", bufs=4))
        npool = ctx.enter_context(tc.tile_pool(name="attn", bufs=1))
        xTp_cm = tc.tile_pool(name="xTp", bufs=1)
        xTp = xTp_cm.__enter__()
        xT = xTp.tile([128, 8 * S], BF16)      # [p, (dt, s)]
        xTv = xT[:, :].rearrange("p (d s) -> p d s", d=8)

        # DRAM bounce buffers for the pairwise output exchange
        dr = ctx.enter_context(tc.tile_pool(name="dr", bufs=1, space="DRAM"))
        ag_in = [dr.tile([JL, 128, 512], BF16, name=f"agin{i}")
                 for i in range(2)]
        ag_out = [dr.tile([2, JL, 128, 512], BF16, name=f"agout{i}")
                  for i in range(2)]

        psm = ctx.enter_context(tc.tile_pool(name="psm", bufs=2, space="PSUM"))
        pso = ctx.enter_context(tc.tile_pool(name="pso", bufs=2, space="PSUM"))

        # bias rows broadcast to all 128 partitions once
        bvb = const.tile([128, 512], BF16)
        bpb = const.tile([128, D], BF16)
        pbias = pso.tile([128, 512], F32, tag="pk", name="pbias")
        nc.tensor.matmul(pbias[:, :], ones1[:, :], bvr[:, :],
                         start=True, stop=True)
        nc.vector.tensor_copy(bvb[:, :], pbias[:, :])
        pbias2 = psm.tile([128, 1024], F32, tag="ps", name="pbias2")
        for nh in range(2):
            nc.tensor.matmul(pbias2[:, nh * 512:(nh + 1) * 512], ones1[:, :],
                             bpr[:, nh * 512:(nh + 1) * 512],
                             start=True, stop=True)
        nc.vector.tensor_copy(bpb[:, :], pbias2[:, :])

        wkq_cm = tc.tile_pool(name="wkq", bufs=1)
        wkq = wkq_cm.__enter__()

        def alloc_w(pool, tag):
            return [pool.tile([128, 512], BF16, tag=tag + "b" + str(dt_),
                              name=tag + str(dt_)) for dt_ in range(8)]

        def load_w_j0(wd, tiles, dt_):
            nc.gpsimd.dma_start(
                out=tiles[dt_][:, 0:128],
                in_=wd[dt_ * 128:(dt_ + 1) * 128, 0:128])

        def load_w_rest(wd, tiles):
            for dt_ in range(8):
                nc.gpsimd.dma_start(
                    out=tiles[dt_][:, 128:512],
                    in_=wd[dt_ * 128:(dt_ + 1) * 128, 128:512])

        wkb = None
        wqb = None

        def k_step(j, sc):
            pkc = pso.tile([128, 512], F32, tag="pk", name=f"pk{j}_{sc}")
            for dt_ in range(8):
                nc.tensor.matmul(
                    pkc[:, :],
                    wkb[dt_][:, j * 128:(j + 1) * 128],
                    xTv[:, dt_, sc * 512:(sc + 1) * 512],
                    start=(dt_ == 0), stop=(dt_ == 7),
                )
            nc.vector.tensor_scalar_add(
                KTv[:, j, sc * 512:(sc + 1) * 512], pkc[:, :],
                bkp[:, j:j + 1])

        def q_step(j, qs):
            pqc = pso.tile([128, 512], F32, tag="pk", name=f"pq{j}_{qs}")
            for dt_ in range(8):
                nc.tensor.matmul(
                    pqc[:, :],
                    wqb[dt_][:, j * 128:(j + 1) * 128],
                    xTv[:, dt_, qs * 512:(qs + 1) * 512],
                    start=(dt_ == 0), stop=(dt_ == 7),
                )
            nc.vector.tensor_scalar_add(
                QTv[:, j, qs * 512:(qs + 1) * 512], pqc[:, :],
                bqp[:, j:j + 1])

        def kq_chunks(j):
            # K (all 4 key chunks) + Q for the first-processed slot only;
            # Q chunks for later slots are deferred into later phases
            return [
                lambda j=j: k_step(j, 0),
                lambda j=j: q_step(j, SLOTS[0]),
                lambda j=j: k_step(j, 1),
                lambda j=j: k_step(j, 2),
                lambda j=j: k_step(j, 3),
            ]

        # ---------------- Phase A: xT via PE transposes ----------------
        # One small j0 weight-column DMA rides behind each x tile (same
        # gpsimd cast queue, ~1/8th the size, so x streaming stays smooth);
        # K0 chunk matmuls are emitted as soon as their x tiles + weights
        # have landed, filling the PE while later x tiles stream in.
        wkb = alloc_w(wkq, "wk")
        wqb = alloc_w(wkq, "wq")
        with tc.tile_pool(name="xstg", bufs=6) as xpool:
            for st in range(16):
                xb16 = xpool.tile([128, D], BF16, tag="xb16")
                nc.gpsimd.dma_start(out=xb16[:, :],
                                    in_=x_d[st * 128:(st + 1) * 128, :])
                if st < 8:
                    load_w_j0(wk_d, wkb, st)
                else:
                    load_w_j0(wq_d, wqb, st - 8)
                pt = psm.tile([128, 1024], BF16, tag="ps", name=f"pt{st}")
                for dt_ in range(8):
                    nc.tensor.transpose(
                        pt[:, dt_ * 128:(dt_ + 1) * 128],
                        xb16[:, dt_ * 128:(dt_ + 1) * 128],
                        ident[:, :],
                    )
                dst = xTv[:, :, st * 128:(st + 1) * 128]
                src = pt[:, :].rearrange("p (d s) -> p d s", d=8)
                if st % 2 == 0:
                    nc.scalar.copy(dst, src)
                else:
                    nc.vector.tensor_copy(dst, src)
                if st == 8:
                    k_step(0, 0)     # x tiles 0-3 + all wk j0 cols ready
                elif st == 10:
                    k_step(0, 1)
                elif st == 12:
                    k_step(0, 2)
        k_step(0, 3)
        q_step(0, SLOTS[0])

        def load_w(wd, pool, tag, cols):
            tiles = []
            for dt_ in range(8):
                wb = pool.tile([128, cols], BF16, tag=tag + "b" + str(dt_))
                nc.gpsimd.dma_start(out=wb[:, :],
                                    in_=wd[dt_ * 128:(dt_ + 1) * 128, :])
                tiles.append(wb)
            return tiles

        rpbs = {}

        def attn_fast_evac(j, qs, poA, poB):
            qsl = slice(qs * 512, (qs + 1) * 512)
            lp = npool.tile([1, 1024], F32, tag="lp", name=f"lp{j}_{qs}")
            nc.vector.tensor_copy(lp[0:1, 0:512], poA[64:65, :])
            nc.vector.tensor_copy(lp[0:1, 512:1024], poB[64:65, :])
            nc.vector.tensor_copy(oTv[0:64, j, qsl], poA[0:64, :])
            nc.vector.tensor_copy(oTv[64:128, j, qsl], poB[0:64, :])
            rp = npool.tile([1, 1024], F32, tag="rp", name=f"rp{j}_{qs}")
            nc.vector.reciprocal_approx_fast(rp[:, :], lp[:, :])
            rpb = npool.tile([1, 1024], BF16, tag="rpb", name=f"rpb{j}_{qs}")
            nc.vector.tensor_copy(rpb[:, :], rp[:, :])
            rpbs[(j, qs)] = rpb

        def attn_norm_tail(j, qs):
            qsl = slice(qs * 512, (qs + 1) * 512)
            rpb = rpbs.pop((j, qs))
            pbc = pso.tile([128, 512], F32, tag="pk", name=f"pbc{j}_{qs}")
            nc.tensor.matmul(pbc[0:64, :], ones1[0:1, 0:64],
                             rpb[0:1, 0:512], start=True, stop=True)
            nc.tensor.matmul(pbc[64:128, :], ones1[0:1, 0:64],
                             rpb[0:1, 512:1024], start=True, stop=True,
                             tile_position=(0, 64))
            rbc = npool.tile([128, 512], F32, tag="rbc", name=f"rbc{j}_{qs}")
            nc.vector.tensor_copy(rbc[:, :], pbc[:, :])
            nc.vector.tensor_tensor(
                oTv[0:64, j, qsl], oTv[0:64, j, qsl], rbc[0:64, :], MULT)
            nc.vector.tensor_tensor(
                oTv[64:128, j, qsl], oTv[64:128, j, qsl], rbc[64:128, :],
                MULT)
            if qs == 2:
                nc.sync.dma_start(out=ag_in[0][j, :, :], in_=oTv[:, j, qsl])
            elif qs == 3:
                nc.sync.dma_start(out=ag_in[1][j, :, :], in_=oTv[:, j, qsl])

        pending_norm = []

        def flush_norm():
            while pending_norm:
                pending_norm.pop(0)()

        def attn_group(j, qs, ktg, poA, poB):
            qsl = slice(qs * 512, (qs + 1) * 512)
            kts = (2 * ktg, 2 * ktg + 1)
            pss = []
            for kt in kts:
                ps = psm.tile([128, 1024], F32, tag="ps",
                              name=f"ps{j}_{qs}_{kt}")
                pss.append(ps)
                nc.tensor.matmul(
                    ps[:, 0:512],
                    KTv[0:64, j, kt * 128:(kt + 1) * 128],
                    QTv[0:64, j, qsl],
                    start=True, stop=True, tile_position=(0, 0))
                nc.tensor.matmul(
                    ps[:, 512:1024],
                    KTv[64:128, j, kt * 128:(kt + 1) * 128],
                    QTv[64:128, j, qsl],
                    start=True, stop=True, tile_position=(64, 0))
            ePs = []
            for ps in pss:
                eP = apool.tile([128, 1024], BF16, tag="eP")
                nc.scalar.activation(eP[:, :], ps[:, :], Exp, scale=0.125)
                ePs.append(eP)
            for kt, eP in zip(kts, ePs):
                nc.tensor.matmul(
                    poA[:, :], Vv[:, kt, 2 * j, 0:65], eP[:, 0:512],
                    start=(kt == 0), stop=(kt == 15))
                nc.tensor.matmul(
                    poB[:, :], Vv[:, kt, 2 * j + 1, 0:65], eP[:, 512:1024],
                    start=(kt == 0), stop=(kt == 15))

        def attn_block(j, qs, interleave=None):
            poA = pso.tile([65, 512], F32, tag="po", name=f"poA{j}_{qs}")
            poB = pso.tile([65, 512], F32, tag="po", name=f"poB{j}_{qs}")
            nsteps = len(interleave) if interleave else 0
            si = 0
            for ktg in range(8):
                attn_group(j, qs, ktg, poA, poB)
                if interleave and si < nsteps and ktg < 6:
                    interleave[si]()
                    si += 1
                if ktg == 1:
                    flush_norm()
            while interleave and si < nsteps:
                interleave[si]()
                si += 1
            attn_fast_evac(j, qs, poA, poB)
            pending_norm.append(lambda j=j, qs=qs: attn_norm_tail(j, qs))

        # --- V proj pipelined with the first attention block (slot 2) ---
        with tc.tile_pool(name="wv", bufs=1) as wvp:
            wvb = load_w(wv_d, wvp, "wv", 512)
            load_w_rest(wk_d, wkb)
            load_w_rest(wq_d, wqb)

            def v_st(st):
                pv = psm.tile([128, 512], F32, tag="ps", name=f"pv{st}")
                for dt_ in range(8):
                    nc.tensor.matmul(
                        pv[:, :],
                        xTv[:, dt_, st * 128:(st + 1) * 128],
                        wvb[dt_][:, :],
                        start=(dt_ == 0), stop=(dt_ == 7),
                    )
                dst = Vv[:, st, :, 0:64]
                src_ = pv[:, :].rearrange("p (h d) -> p h d", h=8)
                bsr = bvb[:, :].rearrange("p (h d) -> p h d", h=8)
                nc.vector.tensor_tensor(dst, src_, bsr, ADD)

            poA0 = pso.tile([65, 512], F32, tag="po", name="poA0_s2")
            poB0 = pso.tile([65, 512], F32, tag="po", name="poB0_s2")
            for stg in range(8):
                v_st(2 * stg)
                v_st(2 * stg + 1)
                attn_group(0, 2, stg, poA0, poB0)
            attn_fast_evac(0, 2, poA0, poB0)
            pending_norm.append(lambda: attn_norm_tail(0, 2))

        # rest of slot-2 phase; block 3 hosts the deferred slot-0 Q chunks
        for step in kq_chunks(1):
            step()
        attn_block(1, 2, interleave=kq_chunks(2))
        attn_block(2, 2, interleave=kq_chunks(3))
        attn_block(3, 2,
                   interleave=[lambda j=j: q_step(j, SLOTS[1])
                               for j in range(JL)])
        flush_norm()

        # exchange #1: slot-2 outputs (this core's q_local 1024:1535)
        nc.gpsimd.collective_compute(
            "AllGather",
            mybir.AluOpType.bypass,
            replica_groups=[[0, 1], [2, 3], [4, 5], [6, 7]],
            ins=[ag_in[0][:, :, :]],
            outs=[ag_out[0][:, :, :, :]],
        )

        def consume_ag(i, qoff):
            stg = npool.tile([128, JL * 512], BF16, tag="agA",
                             name=f"agA{i}")
            stg2 = npool.tile([128, JL * 512], BF16, tag="agB",
                              name=f"agB{i}")
            sv = stg[:, :].rearrange("p (j q) -> p j q", j=JL)
            sv2 = stg2[:, :].rearrange("p (j q) -> p j q", j=JL)
            src0 = ag_out[i][0, :, :, :].rearrange("j p q -> p j q")
            src1 = ag_out[i][1, :, :, :].rearrange("j p q -> p j q")
            nc.sync.dma_start(out=sv, in_=src0)
            nc.sync.dma_start(out=sv2, in_=src1)
            dst = oPv[:, :, qoff:qoff + 512]
            # peer slot = slot1 on even cores (msel col0 = 1), slot0 on odd
            nc.vector.tensor_scalar_mul(dst, sv2, msel[:, 0:1])
            nc.vector.scalar_tensor_tensor(
                dst, sv, msel[:, 1:2], dst, op0=MULT, op1=ADD)

        consume_ag(0, 0)

        # ---------------- slot 0 (own rows, first half) ----------------
        # hosts the remaining deferred Q chunks (xT/wkq still alive here)
        for bj in range(JL):
            attn_block(bj, 0,
                       interleave=[lambda j=bj: q_step(j, SLOTS[2]),
                                   lambda j=bj: q_step(j, SLOTS[3])])
        flush_norm()
        wkq_cm.__exit__(None, None, None)
        xTp_cm.__exit__(None, None, None)

        # ---------------- slot 3 + proj(own first half) -----------------
        with tc.tile_pool(name="wp", bufs=1) as wpp, \
             tc.tile_pool(name="ystg", bufs=2) as ypool:
            wpb = load_w(wp_d, wpp, "wp", D)

            def proj_lhs(c, qt):
                qsl = slice(qt * 128, (qt + 1) * 128)
                if c < 4:
                    return oTv[:, c, qsl]
                return oPv[:, c - 4, qsl]

            def proj(qt):
                py = psm.tile([128, 1024], F32, tag="ps", name=f"py{qt}")
                for c in range(8):
                    for nh in range(2):
                        nc.tensor.matmul(
                            py[:, nh * 512:(nh + 1) * 512],
                            proj_lhs(c, qt),
                            wpb[c][:, nh * 512:(nh + 1) * 512],
                            start=(c == 0), stop=(c == 7),
                        )
                ys = ypool.tile([128, 1024], F32, tag="ys")
                nc.vector.tensor_tensor(ys[:, :], py[:, :], bpb[:, :], ADD)
                nc.sync.dma_start(
                    out=out_d[qt * 128:(qt + 1) * 128, :], in_=ys[:, :])

            def proj_nh(qt, nh):
                ph = pso.tile([128, 512], F32, tag="pk",
                              name=f"ph{qt}_{nh}")
                for c in range(8):
                    nc.tensor.matmul(
                        ph[:, :],
                        proj_lhs(c, qt),
                        wpb[c][:, nh * 512:(nh + 1) * 512],
                        start=(c == 0), stop=(c == 7),
                    )
                ys = ypool.tile([128, 512], F32, tag="ysh")
                nc.vector.tensor_tensor(
                    ys[:, :], ph[:, :], bpb[:, nh * 512:(nh + 1) * 512],
                    ADD)
                nc.sync.dma_start(
                    out=out_d[qt * 128:(qt + 1) * 128,
                              nh * 512:(nh + 1) * 512], in_=ys[:, :])

            # slot 3 blocks: host proj halves for qt 0-1 (deps: slot-0 attn
            # + exchange #1); qt 2-3 halves go into the slot-1 blocks
            halves = [(qt, nh) for qt in range(4) for nh in range(2)]
            for bj in range(JL):
                il = [lambda qt=qt, nh=nh: proj_nh(qt, nh)
                      for (qt, nh) in halves[bj:bj + 1]]
                attn_block(bj, 3, interleave=il)
            flush_norm()

            # exchange #2: slot-3 outputs
            nc.gpsimd.collective_compute(
                "AllGather",
                mybir.AluOpType.bypass,
                replica_groups=[[0, 1], [2, 3], [4, 5], [6, 7]],
                ins=[ag_in[1][:, :, :]],
                outs=[ag_out[1][:, :, :, :]],
            )
            consume_ag(1, 512)

            # ---------------- slot 1 (own rows, second half) -------------
            for bj in range(JL):
                il = [lambda qt=qt, nh=nh: proj_nh(qt, nh)
                      for (qt, nh) in halves[4 + bj:5 + bj]]
                attn_block(bj, 1, interleave=il)

            # tail: qt=4's j!=3 chunks overlap the last block's norm chain
            py4 = psm.tile([128, 1024], F32, tag="ps", name="py4")
            for c in (0, 1, 2, 4, 5, 6, 7):
                for nh in range(2):
                    nc.tensor.matmul(
                        py4[:, nh * 512:(nh + 1) * 512],
                        proj_lhs(c, 4),
                        wpb[c][:, nh * 512:(nh + 1) * 512],
                        start=(c == 0), stop=False,
                    )
            flush_norm()
            for nh in range(2):
                nc.tensor.matmul(
                    py4[:, nh * 512:(nh + 1) * 512],
                    proj_lhs(3, 4),
                    wpb[3][:, nh * 512:(nh + 1) * 512],
                    start=False, stop=True,
                )
            ys4 = ypool.tile([128, 1024], F32, tag="ys")
            nc.vector.tensor_tensor(ys4[:, :], py4[:, :], bpb[:, :], ADD)
            nc.sync.dma_start(out=out_d[4 * 128:5 * 128, :], in_=ys4[:, :])
            for qt in range(5, 8):
                proj(qt)

    nc.finalize()
    return nc


def _in_maps(x, W_qkv, b_qkv, W_proj, b_proj):
    x = np.asarray(x, np.float32)
    W_qkv = np.asarray(W_qkv, np.float32)
    b_qkv = np.asarray(b_qkv, np.float32)
    W_proj = np.ascontiguousarray(np.asarray(W_proj, np.float32))
    b_proj = np.asarray(b_proj, np.float32)
    Wq = W_qkv[:, 0:D]
    Wk = W_qkv[:, D:2 * D]
    Wv = W_qkv[:, 2 * D:3 * D]
    bq, bk = b_qkv[0:D], b_qkv[D:2 * D]
    bv = b_qkv[2 * D:3 * D]
    maps = []
    for c in range(NC_):
        b, hh = c // 2, c % 2
        hs = slice(hh * 512, (hh + 1) * 512)
        xb = np.concatenate(
            [x[b, hh * OWN:(hh + 1) * OWN],
             x[b, (1 - hh) * OWN:(2 - hh) * OWN]], axis=0)
        wp = np.concatenate(
            [W_proj[hh * 512:(hh + 1) * 512],
             W_proj[(1 - hh) * 512:(2 - hh) * 512]], axis=0)
        msel = np.zeros((128, 2), np.float32)
        msel[:, 0] = 1.0 if hh == 0 else 0.0
        msel[:, 1] = 1.0 - msel[:, 0]
        maps.append({
            "xb": np.ascontiguousarray(xb),
            "wq": np.ascontiguousarray(Wq[:, hs]),
            "wk": np.ascontiguousarray(Wk[:, hs]),
            "wv": np.ascontiguousarray(Wv[:, hs]),
            "wp": np.ascontiguousarray(wp),
            "bqp": np.ascontiguousarray(bq[hs].reshape(JL, 128).T),
            "bkp": np.ascontiguousarray(bk[hs].reshape(JL, 128).T),
            "bvr": np.ascontiguousarray(bv[hs].reshape(1, 512)),
            "bpr": np.ascontiguousarray(b_proj.reshape(1, D)),
            "msel": msel,
        })
    return maps


def run(x, W_qkv, b_qkv, W_proj, b_proj, trace=False, tmpdir=None):
    sys.path.insert(0, "/opt/trn_rl_repo")
    from concourse.bass_utils import run_bass_kernel_spmd

    if "nc" not in _cache:
        _cache["nc"] = _build_nc()
    nc = _cache["nc"]
    maps = _in_maps(x, W_qkv, b_qkv, W_proj, b_proj)
    res = run_bass_kernel_spmd(nc, maps, core_ids=list(range(NC_)),
                               trace=trace, tmpdir=tmpdir)
    y = np.empty((B, S, D), np.float32)
    for c in range(NC_):
        b, hh = c // 2, c % 2
        y[b, hh * OWN:(hh + 1) * OWN] = res.results[c]["out"]
    return y, res


def kernel(x, W_qkv, b_qkv, W_proj, b_proj):
    y, _ = run(x, W_qkv, b_qkv, W_proj, b_proj, trace=False)
    return y
